# revision 1
# baseline (speedup 1.0000x reference)
"""BrainRNN Trainium2 kernel: 8-core tensor-parallel Bass/Tile implementation.

Strategy (per sharding hint): shard every weight's output-node dimension (rows
of W, 1024 per layer) across 8 cores -> 128 rows/core.  Each core:
  - streams its W_r / W_h / W_s row-shards (only the structurally non-masked
    column ranges) plus the matching adj column-slices,
  - masks on-chip:  adj int32 -> f32 via cast-DMA (SWDGE), W transposed via
    PE (128x128 tiles), masked = W.T * adj on DVE,
  - matmuls with the stationary operand = masked W.T tile (contraction dim on
    partitions) and the moving operand = h.T / xx.T tiles (batch=32 columns),
  - per layer: sigmoid w/ per-partition bias on ACT, then an 8-core AllGather
    of the (128, 32) xx.T shard -> full (1024, 32) xx.T on every core,
  - final 64-wide output layer computed replicated on every core.

Structural-zero exploitation (shape-derived, not data-dependent):
  Wr_m(k) has columns [: (k+1)*1024] zeroed  -> never load them.
  Ws_m(j) only uses W_s[j][:, : (j+1)*1024]  -> never load the padding.
"""

import sys

sys.path.insert(0, "/opt/trn_rl_repo")

import numpy as np

D = 1024
L = 8
N = 8192
B = 32
P = 128
NC = 8

_CACHE = {}


def _build(spmd=True, reps=1, ag=True, load_adj=True, do_trans=True, do_dve=True, do_mm='all', do_compute=True, mm_bf16=False, mm_dt=None, w_cast=False):
    import concourse.bacc as bacc
    import concourse.tile as tile
    import concourse.mybir as mybir
    import concourse.bass as bass

    F32 = mybir.dt.float32
    I32 = mybir.dt.int32
    if mm_dt is None:
        mm_dt = "bf16" if mm_bf16 else "f32"
    MMDT = {"f32": F32, "bf16": mybir.dt.bfloat16, "f16": mybir.dt.float16}[mm_dt]
    mm_bf16 = MMDT != F32
    if w_cast:
        assert mm_bf16
    WDT = MMDT if w_cast else F32

    nc = bacc.Bacc(
        "TRN2", target_bir_lowering=False, debug=False, num_devices=NC if spmd else 1
    )

    # ---- DRAM I/O ------------------------------------------------------
    x_d = nc.dram_tensor("x", [B, 256], F32, kind="ExternalInput")
    h_d = nc.dram_tensor("h", [B, N], F32, kind="ExternalInput")
    win_d = nc.dram_tensor("win", [P, 256], F32, kind="ExternalInput")
    bin_d = nc.dram_tensor("bin", [P, 1], F32, kind="ExternalInput")
    bh_d = nc.dram_tensor("bh", [P, L - 1], F32, kind="ExternalInput")
    wo_d = nc.dram_tensor("wo", [64, D], F32, kind="ExternalInput")
    bo_d = nc.dram_tensor("bo", [64, 1], F32, kind="ExternalInput")
    id_d = nc.dram_tensor("ident", [P, P], F32, kind="ExternalInput")
    wr_d = [
        nc.dram_tensor(f"wr{k}", [P, (7 - k) * D], F32, kind="ExternalInput")
        for k in range(7)
    ]
    ar_d = [
        nc.dram_tensor(f"ar{k}", [(7 - k) * D, P], I32, kind="ExternalInput")
        for k in range(7)
    ]
    wh_d = [
        nc.dram_tensor(f"wh{i}", [P, D], F32, kind="ExternalInput") for i in range(7)
    ]
    ah_d = [
        nc.dram_tensor(f"ah{i}", [D, P], I32, kind="ExternalInput") for i in range(7)
    ]
    ws_d = [
        nc.dram_tensor(f"ws{j}", [P, (j + 1) * D], F32, kind="ExternalInput")
        for j in range(6)
    ]
    as_d = [
        nc.dram_tensor(f"as{j}", [(j + 1) * D, P], I32, kind="ExternalInput")
        for j in range(6)
    ]
    outT_d = nc.dram_tensor("outT", [64, B], F32, kind="ExternalOutput")

    SIG = mybir.ActivationFunctionType.Sigmoid
    CPY = mybir.ActivationFunctionType.Copy

    with tile.TileContext(nc) as tc:
        with (
            tc.tile_pool(name="cst", bufs=1) as cst,
            tc.tile_pool(name="big", bufs=1) as big,
            tc.tile_pool(name="wrp", bufs=3) as wrp,
            tc.tile_pool(name="arp", bufs=3) as arp,
            tc.tile_pool(name="whp", bufs=4) as whp,
            tc.tile_pool(name="ahp", bufs=4) as ahp,
            tc.tile_pool(name="wsp", bufs=9) as wsp,
            tc.tile_pool(name="asp", bufs=9) as asp,
            tc.tile_pool(name="mqp", bufs=6) as mqp,
            tc.tile_pool(name="xxp", bufs=1) as xxp,
            tc.tile_pool(name="psq", bufs=5, space="PSUM") as psq,
            tc.tile_pool(name="psl", bufs=3, space="PSUM") as psl,
            tc.tile_pool(name="dram", bufs=1, space="DRAM") as dram,
        ):
            # ---- constants / small tensors -----------------------------
            id_sb = cst.tile([P, P], F32, tag="id")
            nc.sync.dma_start(id_sb[:], id_d[:, :])
            if w_cast:
                id16_sb = cst.tile([P, P], MMDT, tag="id16")
                nc.vector.tensor_copy(id16_sb[:], id_sb[:])
            else:
                id16_sb = id_sb
            x_sb = cst.tile([B, 256], F32, tag="x")
            nc.sync.dma_start(x_sb[:], x_d[:, :])
            win_sb = cst.tile([P, 256], F32, tag="win")
            nc.sync.dma_start(win_sb[:], win_d[:, :])
            bin_sb = cst.tile([P, 1], F32, tag="bin")
            nc.sync.dma_start(bin_sb[:], bin_d[:, :])
            bh_sb = cst.tile([P, L - 1], F32, tag="bh")
            nc.sync.dma_start(bh_sb[:], bh_d[:, :])
            wo_sb = cst.tile([64, D], F32, tag="wo")
            nc.sync.dma_start(wo_sb[:], wo_d[:, :])
            bo_sb = cst.tile([64, 1], F32, tag="bo")
            nc.sync.dma_start(bo_sb[:], bo_d[:, :])

            # ---- h -> hT (64 tiles of [128, 32]) -----------------------
            h_sb = big.tile([B, N], F32, tag="h")
            nc.sync.dma_start(h_sb[:], h_d[:, :])
            ht_sb = cst.tile([P, 64 * B], MMDT, tag="ht")
            for q in range(16):  # 4 tiles per psum quad
                tq = psq.tile([P, 4 * B], F32, tag="tq")
                for t4 in range(4):
                    t = q * 4 + t4
                    nc.tensor.transpose(
                        tq[:, t4 * B : (t4 + 1) * B],
                        h_sb[:, t * P : (t + 1) * P],
                        id_sb[:B, :B],
                    )
                nc.vector.tensor_copy(ht_sb[:, q * 4 * B : (q + 1) * 4 * B], tq[:])

            # ---- x -> xT (2 tiles), W_in -> W_inT (2 tiles) ------------
            xt_sb = cst.tile([P, 2 * B], MMDT, tag="xt")
            winT_sb = cst.tile([P, 256], MMDT, tag="winT")
            tq = psq.tile([P, 512], F32, tag="tq", name="tqx")
            nc.tensor.transpose(tq[:, 0:B], x_sb[:, 0:P], id_sb[:B, :B])
            nc.tensor.transpose(tq[:, B : 2 * B], x_sb[:, P : 2 * P], id_sb[:B, :B])
            nc.vector.tensor_copy(xt_sb[:], tq[:, 0 : 2 * B])
            tq = psq.tile([P, 512], F32, tag="tq")
            nc.tensor.transpose(tq[:, 0:P], win_sb[:, 0:P], id_sb[:, :])
            nc.tensor.transpose(tq[:, P : 2 * P], win_sb[:, P : 2 * P], id_sb[:, :])
            nc.vector.tensor_copy(winT_sb[:], tq[:, 0 : 2 * P])

            # ---- per-layer state ---------------------------------------
            xxT = [None] * L  # gathered xx.T per layer: [128, 8*32]

            class Acc:
                def __init__(self, total):
                    self.ps = psl.tile([P, B], F32, tag="lps")
                    self.n = 0
                    self.total = total

                def mm(self, lhsT, rhs):
                    if not do_compute:
                        self.n += 1
                        return
                    if do_mm == "all":
                        nc.tensor.matmul(
                            self.ps[:, :],
                            lhsT,
                            rhs,
                            start=(self.n == 0),
                            stop=(self.n == self.total - 1),
                        )
                    elif self.n == 0:
                        nc.tensor.matmul(
                            self.ps[:, :], lhsT, rhs, start=True, stop=True
                        )
                    self.n += 1

            def masked_mms(acc, w_slab, a_slab, ntiles, rhs_of):
                """w_slab [128, ntiles*128] natural (d rows, n cols);
                a_slab [128, ntiles*128] f32, a_slab[p, t*128+d] = adj[n0+t*128+p, d];
                emits per tile: PE transpose, DVE mask (per quad), matmul."""
                assert ntiles % 4 == 0
                if not do_compute:
                    return
                if not do_dve:
                    assert not do_trans
                    for t in range(ntiles):
                        acc.mm(w_slab[:, t * P : (t + 1) * P], rhs_of(t))
                    return
                for q in range(ntiles // 4):
                    mq = mqp.tile([P, 512], MMDT, tag="mq")
                    if do_trans:
                        tq = psq.tile([P, 512], WDT, tag="tq")
                        for t4 in range(4):
                            t = q * 4 + t4
                            nc.tensor.transpose(
                                tq[:, t4 * P : (t4 + 1) * P],
                                w_slab[:, t * P : (t + 1) * P],
                                id16_sb[:, :] if w_cast else id_sb[:, :],
                            )
                        if a_slab is not None:
                            nc.vector.tensor_mul(
                                mq[:], tq[:], a_slab[:, q * 512 : (q + 1) * 512]
                            )
                        else:
                            nc.vector.tensor_copy(mq[:], tq[:])
                    else:
                        if a_slab is not None:
                            nc.vector.tensor_mul(
                                mq[:],
                                w_slab[:, q * 512 : (q + 1) * 512],
                                a_slab[:, q * 512 : (q + 1) * 512],
                            )
                        else:
                            nc.vector.tensor_copy(
                                mq[:], w_slab[:, q * 512 : (q + 1) * 512]
                            )
                    for t4 in range(4):
                        t = q * 4 + t4
                        acc.mm(mq[:, t4 * P : (t4 + 1) * P], rhs_of(t))

            def emit_wr(acc, k):
                """stream W_r[k] shard in 2 half-slabs, accumulate R_k into acc."""
                T = (7 - k) * 8  # 128-tiles total
                for half in range(2):
                    Th = T // 2
                    w_sl = wrp.tile([P, Th * P], WDT, tag="wr")
                    (nc.gpsimd if w_cast else nc.sync).dma_start(
                        w_sl[:], wr_d[k][:, half * Th * P : (half + 1) * Th * P]
                    )
                    if load_adj:
                        a_sl = arp.tile([P, Th * P], WDT, tag="ar")
                        nc.gpsimd.dma_start(
                            a_sl[:].rearrange("p (t d) -> p t d", t=Th),
                            ar_d[k][half * Th * P : (half + 1) * Th * P, :].rearrange(
                                "(t p) d -> p t d", p=P
                            ),
                        )
                    else:
                        a_sl = None
                    base = (k + 1) * 8 + half * Th  # global 128-tile index into hT
                    masked_mms(
                        acc,
                        w_sl,
                        a_sl,
                        Th,
                        lambda t, base=base: ht_sb[
                            :, (base + t) * B : (base + t + 1) * B
                        ],
                    )

            def emit_block(acc, w_dram, a_dram, col0, row0, xxT_l):
                """one [128 x 1024] W block (cols col0..col0+1024 of w_dram) masked by
                adj rows row0..row0+1024 of a_dram, matmul'd against xxT_l tiles."""
                w_sl = (whp if xxT_l is None else wsp).tile([P, D], WDT, tag="wb")
                (nc.gpsimd if w_cast else nc.sync).dma_start(
                    w_sl[:], w_dram[:, col0 : col0 + D]
                )
                if load_adj:
                    a_sl = (ahp if xxT_l is None else asp).tile([P, D], WDT, tag="ab")
                    nc.gpsimd.dma_start(
                        a_sl[:].rearrange("p (t d) -> p t d", t=8),
                        a_dram[row0 : row0 + D, :].rearrange("(t p) d -> p t d", p=P),
                    )
                else:
                    a_sl = None
                return w_sl, a_sl

            def finalize(l, acc):
                """sigmoid(+bias), allgather, reload gathered xxT."""
                if not do_compute:
                    return
                xs = cst.tile([P, B], MMDT, tag="xshard", name=f"xs{l}")
                bias = bin_sb[:, 0:1] if l == 0 else bh_sb[:, l - 1 : l]
                nc.scalar.activation(xs[:], acc.ps[:, :], SIG, bias=bias, scale=1.0)
                cci = dram.tile([P, B], MMDT, tag=f"cci{l}", name=f"cci{l}")
                cco = dram.tile([NC * P, B], MMDT, tag=f"cco{l}", name=f"cco{l}")
                nc.sync.dma_start(cci[:], xs[:])
                if spmd and ag:
                    nc.gpsimd.collective_compute(
                        "AllGather",
                        mybir.AluOpType.bypass,
                        replica_groups=[list(range(NC))],
                        ins=[cci[:].opt()],
                        outs=[cco[:].opt()],
                    )
                else:
                    # timing-only stand-in for the AllGather bounce
                    nc.sync.dma_start(cco[0:P, :], cci[:])
                xxT[l] = xxp.tile([P, 8 * B], MMDT, tag=f"xxT{l}", name=f"xxT{l}")
                eng = nc.sync
                eng.dma_start(
                    xxT[l][:].rearrange("p (t b) -> p t b", t=8),
                    cco[:].rearrange("(t p) b -> p t b", p=P),
                )

            # ---------------- layer 0 -----------------------------------
            n_mms = [0] * L
            n_mms[0] = 2 + 56
            for l in range(1, L):
                n_mms[l] = 8 + (8 * (l - 1) if l >= 2 else 0) + ((7 - l) * 8 if l <= 6 else 0)

            for _rep in range(reps):
                acc = Acc(n_mms[0])
                acc.mm(winT_sb[:, 0:P], xt_sb[:, 0:B])
                acc.mm(winT_sb[:, P : 2 * P], xt_sb[:, B : 2 * B])
                emit_wr(acc, 0)
                finalize(0, acc)

                # ---------------- layers 1..7 -------------------------------
                for l in range(1, L):
                    acc = Acc(n_mms[l])
                    # hidden term: W_h[l-1] masked, vs xxT[l-1]
                    w_sl, a_sl = emit_block(acc, wh_d[l - 1], ah_d[l - 1], 0, 0, None)
                    masked_mms(
                        acc, w_sl, a_sl, 8,
                        lambda t: xxT[l - 1][:, t * B : (t + 1) * B],
                    )
                    # skip term: W_s[l-2] blocks 0..l-2 vs xxT[mb]
                    if l >= 2:
                        j = l - 2
                        for mb in range(l - 1):
                            w_sl, a_sl = emit_block(
                                acc, ws_d[j], as_d[j], mb * D, mb * D, xxT[mb]
                            )
                            masked_mms(
                                acc, w_sl, a_sl, 8,
                                lambda t, mb=mb: xxT[mb][:, t * B : (t + 1) * B],
                            )
                    # recurrent term R_l
                    if l <= 6:
                        emit_wr(acc, l)
                    finalize(l, acc)

                # ---------------- output layer ------------------------------
                if not do_compute:
                    tout = cst.tile([64, B], F32, tag="outT", name="outTtriv")
                    nc.sync.dma_start(tout[:], win_sb[:64, :B])
                    nc.sync.dma_start(outT_d[:, :], tout[:])
                    continue
                woT_sb = cst.tile([P, 8 * 64], MMDT, tag="woT")
                for q in range(2):
                    tq = psq.tile([P, 512], F32, tag="tq")
                    for t4 in range(4):
                        t = q * 4 + t4
                        nc.tensor.transpose(
                            tq[:, t4 * 64 : (t4 + 1) * 64],
                            wo_sb[:, t * P : (t + 1) * P],
                            id_sb[:64, :64],
                        )
                    nc.vector.tensor_copy(woT_sb[:, q * 256 : (q + 1) * 256], tq[:, 0:256])
                ops = psl.tile([P, B], F32, tag="lps")
                for t in range(8):
                    nc.tensor.matmul(
                        ops[:64, :],
                        woT_sb[:, t * 64 : (t + 1) * 64],
                        xxT[7][:, t * B : (t + 1) * B],
                        start=(t == 0),
                        stop=(t == 7),
                    )
                outT_sb = cst.tile([64, B], F32, tag="outT")
                nc.vector.tensor_scalar_add(outT_sb[:], ops[:64, :], bo_sb[:, 0:1])
                nc.sync.dma_start(outT_d[:, :], outT_sb[:])

    nc.compile()
    return nc


def _shard_inputs(inputs):
    x = np.ascontiguousarray(inputs["x"], dtype=np.float32)
    h = np.ascontiguousarray(inputs["hidden_states"], dtype=np.float32)
    adj = np.asarray(inputs["adj"])
    W_in = np.asarray(inputs["W_in"], dtype=np.float32)
    b_in = np.asarray(inputs["b_in"], dtype=np.float32)
    W_h = np.asarray(inputs["W_h"], dtype=np.float32)
    b_h = np.asarray(inputs["b_h"], dtype=np.float32)
    W_r = np.asarray(inputs["W_r"], dtype=np.float32)
    W_s = np.asarray(inputs["W_s"], dtype=np.float32)
    W_o = np.asarray(inputs["W_o"], dtype=np.float32)
    b_o = np.asarray(inputs["b_o"], dtype=np.float32)
    ident = np.eye(P, dtype=np.float32)

    maps = []
    for c in range(NC):
        sl = slice(c * P, (c + 1) * P)
        m = {
            "x": x,
            "h": h,
            "ident": ident,
            "win": np.ascontiguousarray(W_in[sl]),
            "bin": np.ascontiguousarray(b_in[sl]).reshape(P, 1),
            "bh": np.ascontiguousarray(b_h[:, sl].T),
            "wo": np.ascontiguousarray(W_o),
            "bo": np.ascontiguousarray(b_o).reshape(64, 1),
        }
        for k in range(7):
            m[f"wr{k}"] = np.ascontiguousarray(W_r[k][sl, (k + 1) * D :])
            m[f"ar{k}"] = np.ascontiguousarray(
                adj[(k + 1) * D :, k * D + c * P : k * D + (c + 1) * P],
                dtype=np.int32,
            )
        for i in range(7):
            m[f"wh{i}"] = np.ascontiguousarray(W_h[i][sl])
            m[f"ah{i}"] = np.ascontiguousarray(
                adj[i * D : (i + 1) * D, (i + 1) * D + c * P : (i + 1) * D + (c + 1) * P],
                dtype=np.int32,
            )
        for j in range(6):
            m[f"ws{j}"] = np.ascontiguousarray(W_s[j][sl, : (j + 1) * D])
            m[f"as{j}"] = np.ascontiguousarray(
                adj[: (j + 1) * D, (j + 2) * D + c * P : (j + 2) * D + (c + 1) * P],
                dtype=np.int32,
            )
        maps.append(m)
    return maps


def get_compiled():
    # fp16 matmul operands (f32 PSUM accumulation): ~2.9e-4..6e-4 relative
    # error vs the f32 reference, ~1.5x faster than full-f32 matmuls.
    if "nc" not in _CACHE:
        _CACHE["nc"] = _build(mm_dt="f16", w_cast=True)
    return _CACHE["nc"]


def run(inputs, **run_kwargs):
    from concourse import bass_utils

    nc = get_compiled()
    in_maps = _shard_inputs(inputs)
    res = bass_utils.run_bass_kernel_spmd(
        nc, in_maps, core_ids=list(range(NC)), **run_kwargs
    )
    out = np.ascontiguousarray(res.results[0]["outT"].T.astype(np.float32))
    return out, res


def kernel(**inputs):
    out, _ = run(inputs)
    return out



# revision 2
# speedup vs baseline: 2.8298x; 2.8298x over previous
"""BrainRNN Trainium2 kernel: 8-core tensor-parallel Bass/Tile implementation.

Strategy (per sharding hint): shard every weight's output-node dimension (rows
of W, 1024 per layer) across 8 cores -> 128 rows/core.  Host-side staging does
the sharding *and* the layout work: every weight shard is pre-transposed into
the exact lhsT tile layout the PE consumes ([128 contraction partitions x
128-output-col tiles]), pre-tiled so each DMA is a fully contiguous >=2KB/
partition stream, and cast to f16.  The adjacency slices are staged the same
way (int -> f16 0/1).  On device, each 8-tile chunk is: two contiguous HWDGE
loads (W on the SP ring, adj on the ACT ring), one DVE mask-multiply
(f16 2x mode), and eight 128x128xB matmuls accumulated into the layer's PSUM
tile.  No PE transposes, no SWDGE cast-DMAs anywhere.

Per layer: sigmoid w/ per-partition bias on ACT, 8-core AllGather of the
(128, 32) f16 xx.T shard -> full (1024, 32) xx.T on every core.  The
gather-independent terms (recurrent from h, old skip blocks) are emitted
first and the hidden term (which needs the freshest gather) last, so DMA/DVE/
PE keep streaming underneath the collective's latency.

Structural-zero exploitation (shape-derived, not data-dependent):
  Wr_m(k) has columns [: (k+1)*1024] zeroed  -> never load them.
  Ws_m(j) only uses W_s[j][:, : (j+1)*1024]  -> never load the padding.
"""

import sys

sys.path.insert(0, "/opt/trn_rl_repo")

import numpy as np

D = 1024
L = 8
N = 8192
B = 32
P = 128
NC = 8

_CACHE = {}


def _build(spmd=True, reps=1, ag=True, load_adj=True):
    import concourse.bacc as bacc
    import concourse.tile as tile
    import concourse.mybir as mybir

    F32 = mybir.dt.float32
    F16 = mybir.dt.float16

    nc = bacc.Bacc(
        "TRN2", target_bir_lowering=False, debug=False, num_devices=NC if spmd else 1
    )

    # ---- DRAM I/O (all pre-transposed / pre-tiled / f16 on host) -------
    ht_d = nc.dram_tensor("ht", [P, 64 * B], F16, kind="ExternalInput")
    xt_d = nc.dram_tensor("xt", [P, 2 * B], F16, kind="ExternalInput")
    winT_d = nc.dram_tensor("winT", [P, 256], F16, kind="ExternalInput")
    bin_d = nc.dram_tensor("bin", [P, 1], F32, kind="ExternalInput")
    bh_d = nc.dram_tensor("bh", [P, L - 1], F32, kind="ExternalInput")
    woT_d = nc.dram_tensor("woT", [P, 8 * 64], F16, kind="ExternalInput")
    bo_d = nc.dram_tensor("bo", [64, 1], F32, kind="ExternalInput")
    wr_d = [
        nc.dram_tensor(f"wr{k}", [P, (7 - k) * D], F16, kind="ExternalInput")
        for k in range(7)
    ]
    ar_d = [
        nc.dram_tensor(f"ar{k}", [P, (7 - k) * D], F16, kind="ExternalInput")
        for k in range(7)
    ]
    wh_d = [
        nc.dram_tensor(f"wh{i}", [P, D], F16, kind="ExternalInput") for i in range(7)
    ]
    ah_d = [
        nc.dram_tensor(f"ah{i}", [P, D], F16, kind="ExternalInput") for i in range(7)
    ]
    ws_d = [
        nc.dram_tensor(f"ws{j}", [P, (j + 1) * D], F16, kind="ExternalInput")
        for j in range(6)
    ]
    as_d = [
        nc.dram_tensor(f"as{j}", [P, (j + 1) * D], F16, kind="ExternalInput")
        for j in range(6)
    ]
    outT_d = nc.dram_tensor("outT", [64, B], F32, kind="ExternalOutput")

    SIG = mybir.ActivationFunctionType.Sigmoid

    with tile.TileContext(nc) as tc:
        with (
            tc.tile_pool(name="cst", bufs=1) as cst,
            tc.tile_pool(name="wch", bufs=14) as wch,
            tc.tile_pool(name="ach", bufs=14) as ach,
            tc.tile_pool(name="mqp", bufs=10) as mqp,
            tc.tile_pool(name="xsp", bufs=2) as xsp,
            tc.tile_pool(name="xxp", bufs=1) as xxp,
            tc.tile_pool(name="psl", bufs=4, space="PSUM") as psl,
            tc.tile_pool(name="dram", bufs=1, space="DRAM") as dram,
        ):
            # ---- resident constants ------------------------------------
            ht_sb = cst.tile([P, 64 * B], F16, tag="ht")
            nc.sync.dma_start(ht_sb[:], ht_d[:, :])
            xt_sb = cst.tile([P, 2 * B], F16, tag="xt")
            nc.sync.dma_start(xt_sb[:], xt_d[:, :])
            winT_sb = cst.tile([P, 256], F16, tag="winT")
            nc.sync.dma_start(winT_sb[:], winT_d[:, :])
            bin_sb = cst.tile([P, 1], F32, tag="bin")
            nc.sync.dma_start(bin_sb[:], bin_d[:, :])
            bh_sb = cst.tile([P, L - 1], F32, tag="bh")
            nc.sync.dma_start(bh_sb[:], bh_d[:, :])
            woT_sb = cst.tile([P, 8 * 64], F16, tag="woT")
            nc.sync.dma_start(woT_sb[:], woT_d[:, :])
            bo_sb = cst.tile([64, 1], F32, tag="bo")
            nc.sync.dma_start(bo_sb[:], bo_d[:, :])

            xxT = [None] * L  # gathered xx.T per layer: [128, 8*32] f16

            class Acc:
                def __init__(self, total):
                    self.ps = psl.tile([P, B], F32, tag="lps")
                    self.n = 0
                    self.total = total

                def mm(self, lhsT, rhs):
                    nc.tensor.matmul(
                        self.ps[:, :],
                        lhsT,
                        rhs,
                        start=(self.n == 0),
                        stop=(self.n == self.total - 1),
                    )
                    self.n += 1

            def chunk(acc, w_dram, a_dram, off, rhs_of):
                """one 8-tile chunk: cols [off*D, (off+1)*D) of the slab."""
                w_sl = wch.tile([P, D], F16, tag="w")
                nc.sync.dma_start(w_sl[:], w_dram[:, off * D : (off + 1) * D])
                if load_adj:
                    a_sl = ach.tile([P, D], F16, tag="a")
                    nc.scalar.dma_start(a_sl[:], a_dram[:, off * D : (off + 1) * D])
                    mq = mqp.tile([P, D], F16, tag="mq")
                    nc.vector.tensor_mul(mq[:], w_sl[:], a_sl[:])
                else:
                    mq = w_sl
                for t in range(8):
                    acc.mm(mq[:, t * P : (t + 1) * P], rhs_of(t))

            def finalize(l, acc):
                """sigmoid(+bias), allgather, reload gathered xxT."""
                xs = xsp.tile([P, B], F16, tag="xs")
                bias = bin_sb[:, 0:1] if l == 0 else bh_sb[:, l - 1 : l]
                nc.scalar.activation(xs[:], acc.ps[:, :], SIG, bias=bias, scale=1.0)
                cci = dram.tile([P, B], F16, tag=f"cci{l}", name=f"cci{l}")
                cco = dram.tile([NC * P, B], F16, tag=f"cco{l}", name=f"cco{l}")
                nc.sync.dma_start(cci[:], xs[:])
                if spmd and ag:
                    nc.gpsimd.collective_compute(
                        "AllGather",
                        mybir.AluOpType.bypass,
                        replica_groups=[list(range(NC))],
                        ins=[cci[:].opt()],
                        outs=[cco[:].opt()],
                    )
                else:
                    # timing-only stand-in for the AllGather bounce
                    nc.sync.dma_start(cco[0:P, :], cci[:])
                xxT[l] = xxp.tile([P, 8 * B], F16, tag=f"xxT{l}", name=f"xxT{l}")
                nc.sync.dma_start(
                    xxT[l][:].rearrange("p (t b) -> p t b", t=8),
                    cco[:].rearrange("(t p) b -> p t b", p=P),
                )

            for _rep in range(reps):
                # ---------------- layer 0 -------------------------------
                acc = Acc(58)
                acc.mm(winT_sb[:, 0:P], xt_sb[:, 0:B])
                acc.mm(winT_sb[:, P : 2 * P], xt_sb[:, B : 2 * B])
                for q in range(7):
                    base = 8 + q * 8
                    chunk(
                        acc, wr_d[0], ar_d[0], q,
                        lambda t, base=base: ht_sb[:, (base + t) * B : (base + t + 1) * B],
                    )
                finalize(0, acc)

                # ---------------- layers 1..7 ---------------------------
                for l in range(1, L):
                    acc = Acc(56)
                    # recurrent term (depends only on h) first
                    if l <= 6:
                        for q in range(7 - l):
                            base = (l + 1) * 8 + q * 8
                            chunk(
                                acc, wr_d[l], ar_d[l], q,
                                lambda t, base=base: ht_sb[
                                    :, (base + t) * B : (base + t + 1) * B
                                ],
                            )
                    # skip terms (xxT[mb], all gathered >=1 layer ago)
                    if l >= 2:
                        j = l - 2
                        for mb in range(l - 1):
                            chunk(
                                acc, ws_d[j], as_d[j], mb,
                                lambda t, mb=mb: xxT[mb][:, t * B : (t + 1) * B],
                            )
                    # hidden term (needs the freshest gather) last
                    chunk(
                        acc, wh_d[l - 1], ah_d[l - 1], 0,
                        lambda t: xxT[l - 1][:, t * B : (t + 1) * B],
                    )
                    finalize(l, acc)

                # ---------------- output layer --------------------------
                ops = psl.tile([P, B], F32, tag="ops")
                for t in range(8):
                    nc.tensor.matmul(
                        ops[:64, :],
                        woT_sb[:, t * 64 : (t + 1) * 64],
                        xxT[7][:, t * B : (t + 1) * B],
                        start=(t == 0),
                        stop=(t == 7),
                    )
                outT_sb = cst.tile([64, B], F32, tag="outT")
                nc.vector.tensor_scalar_add(outT_sb[:], ops[:64, :], bo_sb[:, 0:1])
                nc.sync.dma_start(outT_d[:, :], outT_sb[:])

    nc.compile()
    return nc


def _tilT(A):
    """natural W shard [d, n] -> lhsT slab [p, t*d], out[p, t*d+dd] = A[dd, t*128+p]."""
    d, n = A.shape
    T = n // P
    return np.ascontiguousarray(
        A.reshape(d, T, P).transpose(2, 1, 0).reshape(P, T * d).astype(np.float16)
    )


def _tilM(M):
    """mask/activation slice [n, d] -> slab [p, t*d], out[p, t*d+dd] = M[t*128+p, dd]."""
    n, d = M.shape
    T = n // P
    return np.ascontiguousarray(
        M.reshape(T, P, d).transpose(1, 0, 2).reshape(P, T * d).astype(np.float16)
    )


def _shard_inputs(inputs):
    x = np.asarray(inputs["x"], dtype=np.float32)
    h = np.asarray(inputs["hidden_states"], dtype=np.float32)
    adj = np.asarray(inputs["adj"])
    W_in = np.asarray(inputs["W_in"], dtype=np.float32)
    b_in = np.asarray(inputs["b_in"], dtype=np.float32)
    W_h = np.asarray(inputs["W_h"], dtype=np.float32)
    b_h = np.asarray(inputs["b_h"], dtype=np.float32)
    W_r = np.asarray(inputs["W_r"], dtype=np.float32)
    W_s = np.asarray(inputs["W_s"], dtype=np.float32)
    W_o = np.asarray(inputs["W_o"], dtype=np.float32)
    b_o = np.asarray(inputs["b_o"], dtype=np.float32)

    adj16 = adj.astype(np.float16)
    ht = _tilM(h.T)
    xt = _tilM(x.T)
    woT = _tilT(W_o)
    bo = np.ascontiguousarray(b_o).reshape(64, 1)

    maps = []
    for c in range(NC):
        sl = slice(c * P, (c + 1) * P)
        m = {
            "ht": ht,
            "xt": xt,
            "winT": _tilT(W_in[sl]),
            "bin": np.ascontiguousarray(b_in[sl]).reshape(P, 1),
            "bh": np.ascontiguousarray(b_h[:, sl].T),
            "woT": woT,
            "bo": bo,
        }
        for k in range(7):
            m[f"wr{k}"] = _tilT(W_r[k][sl, (k + 1) * D :])
            m[f"ar{k}"] = _tilM(adj16[(k + 1) * D :, k * D + c * P : k * D + (c + 1) * P])
        for i in range(7):
            m[f"wh{i}"] = _tilT(W_h[i][sl])
            m[f"ah{i}"] = _tilM(
                adj16[i * D : (i + 1) * D, (i + 1) * D + c * P : (i + 1) * D + (c + 1) * P]
            )
        for j in range(6):
            m[f"ws{j}"] = _tilT(W_s[j][sl, : (j + 1) * D])
            m[f"as{j}"] = _tilM(
                adj16[: (j + 1) * D, (j + 2) * D + c * P : (j + 2) * D + (c + 1) * P]
            )
        maps.append(m)
    return maps


def get_compiled():
    if "nc" not in _CACHE:
        _CACHE["nc"] = _build()
    return _CACHE["nc"]


def run(inputs, **run_kwargs):
    from concourse import bass_utils

    nc = get_compiled()
    in_maps = _shard_inputs(inputs)
    res = bass_utils.run_bass_kernel_spmd(
        nc, in_maps, core_ids=list(range(NC)), **run_kwargs
    )
    out = np.ascontiguousarray(res.results[0]["outT"].T.astype(np.float32))
    return out, res


def kernel(**inputs):
    out, _ = run(inputs)
    return out
